# revision 33
# baseline (speedup 1.0000x reference)
"""AttentionBlock kernel for TRN2, 8 NeuronCores, data-parallel over batch.

Per core: 2 batch elements of [N=2048, D=128] attention:
  eq = Q@Wq.T+bq ; ek = K@Wk.T+bk ; ev = K@Wv.T+bv
  S  = eq@ek.T/sqrt(D); masked where padding_mask==0 with key_pad
  P  = softmax_m(S) * q_pad[m] ; out = P@ev + Q

Mapping:
 - projections / scores via fp32r matmuls (d on partitions)
 - key-padding mask folded into ek columns (zero masked cols) so masked
   scores are exactly 0; exp(0)=1 corrections folded into rank-1 hvec and
   a scalar hn added to the softmax denominator.
 - exp + row-sum fused on ACT (accum_out); probs written bf16
 - probs DMA-transposed (bf16 xbar) for the AV matmul
 - 1/rowsum folded into the PSUM->SBUF output copy; +Q residual on DVE
"""

import os
import sys

sys.path.insert(0, "/opt/trn_rl_repo")

import numpy as np

import concourse.bass as bass
import concourse.bacc as bacc_mod
import concourse.mybir as mybir
from concourse.tile import TileContext
from concourse.masks import make_identity
from concourse import bass_utils

B, N, D = 16, 2048, 128
NCORES = 8
BPC = B // NCORES  # batches per core
P = 128
NBLK = N // P  # 16
F32 = mybir.dt.float32
F32R = mybir.dt.float32r
BF16 = mybir.dt.bfloat16
I32 = mybir.dt.int32
SCALE = 1.0 / float(np.sqrt(D))

_NC_CACHE = {}


def build_nc():
    nc = bacc_mod.Bacc("TRN2", target_bir_lowering=False)

    q_d = nc.dram_tensor("queries", [BPC, N, D], F32, kind="ExternalInput")
    k_d = nc.dram_tensor("keys", [BPC, N, D], F32, kind="ExternalInput")
    m_d = nc.dram_tensor("padding_mask", [BPC, N], I32, kind="ExternalInput")
    wq_d = nc.dram_tensor("Wq", [D, D], F32, kind="ExternalInput")
    wk_d = nc.dram_tensor("Wk", [D, D], F32, kind="ExternalInput")
    wv_d = nc.dram_tensor("Wv", [D, D], F32, kind="ExternalInput")
    bq_d = nc.dram_tensor("bq", [D], F32, kind="ExternalInput")
    bk_d = nc.dram_tensor("bk", [D], F32, kind="ExternalInput")
    bv_d = nc.dram_tensor("bv", [D], F32, kind="ExternalInput")
    o_d = nc.dram_tensor("out", [BPC, N, D], F32, kind="ExternalOutput")

    with TileContext(nc) as tc:
        with (
            tc.tile_pool(name="const", bufs=1) as cpool,
            tc.tile_pool(name="qk", bufs=2) as qkpool,
            tc.tile_pool(name="qkt", bufs=2) as qktpool,
            tc.tile_pool(name="proj", bufs=2) as projpool,
            tc.tile_pool(name="evp", bufs=2) as evpool,
            tc.tile_pool(name="pblk", bufs=5) as ppool,
            tc.tile_pool(name="pt", bufs=5) as ptpool,
            tc.tile_pool(name="rows", bufs=1) as rowpool,
            tc.tile_pool(name="small", bufs=2) as smpool,
            tc.tile_pool(name="outs", bufs=4) as opool,
            tc.tile_pool(name="ps_big", bufs=2, space="PSUM") as ps_big,
            tc.tile_pool(name="ps_sm", bufs=4, space="PSUM") as ps_sm,
        ):
            # ---------------- setup (once) ----------------
            ident = cpool.tile([P, P], F32)
            make_identity(nc, ident)
            ones_row = cpool.tile([1, P], F32)  # K=1 matmul lhsT
            nc.vector.memset(ones_row, 1.0)
            ones_col = cpool.tile([P, 1], F32)
            nc.vector.memset(ones_col, 1.0)

            # weight transposes WxT[d, d'] = Wx[d', d] via PE transpose
            wts = {}
            for nm, wd in (("wq", wq_d), ("wk", wk_d), ("wv", wv_d)):
                w_nat = cpool.tile([P, P], F32, tag="wnat_" + nm)
                nc.sync.dma_start(w_nat, wd[:, :])
                w_ps = ps_sm.tile([P, P], F32, tag="sm")
                nc.tensor.transpose(w_ps, w_nat, ident)
                w_t = cpool.tile([P, P], F32R, tag="wt_" + nm)
                nc.vector.tensor_copy(w_t, w_ps)
                wts[nm] = w_t
            dps = ps_sm.tile([1, 1], F32, tag="sm")
            nc.tensor.matmul(dps, ident[:, 0:1], ident[:, 0:1], start=True, stop=True)
            bq_c = cpool.tile([P, 1], F32)
            nc.sync.dma_start(bq_c, bq_d[:, None])
            bq_cs = cpool.tile([P, 1], F32)
            nc.vector.tensor_scalar_mul(bq_cs, bq_c, SCALE)
            bk_c = cpool.tile([P, 1], F32)
            nc.sync.dma_start(bk_c, bk_d[:, None])
            bv_r = cpool.tile([1, P], F32)
            nc.sync.dma_start(bv_r, bv_d[None, :])

            for b in range(BPC):
                # ---------------- loads ----------------
                q_sb = qkpool.tile([P, NBLK, P], F32, tag="q")
                nc.gpsimd.dma_start(q_sb, q_d[b].rearrange("(a p) d -> p a d", p=P))
                k_sb = qkpool.tile([P, NBLK, P], F32, tag="k")
                nc.sync.dma_start(k_sb, k_d[b].rearrange("(a p) d -> p a d", p=P))
                mrow_i = rowpool.tile([1, N], I32, tag="mrow_i")
                nc.gpsimd.dma_start(mrow_i, m_d[b][None, :])
                mcol_i = smpool.tile([P, NBLK], I32, tag="mcol_i")
                nc.sync.dma_start(mcol_i, m_d[b].rearrange("(a p) -> p a", p=P))

                # mask rows/cols as fp32; sel = mask!=0 ; msk0 = mask==0
                mrow_f = rowpool.tile([1, N], F32, tag="mrow_f")
                nc.vector.tensor_copy(mrow_f, mrow_i)
                selrow = rowpool.tile([1, N], F32, tag="selrow")
                nc.vector.tensor_scalar(
                    selrow, mrow_f, 0.0, None, mybir.AluOpType.not_equal
                )
                mcol_f = smpool.tile([P, NBLK], F32, tag="mcol_f")
                nc.vector.tensor_copy(mcol_f, mcol_i)
                selcol = smpool.tile([P, NBLK], F32, tag="selcol")
                nc.vector.tensor_scalar(
                    selcol, mcol_f, 0.0, None, mybir.AluOpType.not_equal
                )
                msk0col = smpool.tile([P, NBLK], F32, tag="msk0col")
                nc.vector.tensor_scalar(
                    msk0col, mcol_f, 0.0, None, mybir.AluOpType.is_equal
                )

                # row sums of Q and K (for q_pad / key_pad)
                qs = smpool.tile([P, NBLK], F32, tag="qs")
                nc.vector.reduce_sum(qs, q_sb, axis=mybir.AxisListType.X)
                ks = smpool.tile([P, NBLK], F32, tag="ks")
                nc.vector.reduce_sum(ks, k_sb, axis=mybir.AxisListType.X)
                qp = smpool.tile([P, NBLK], F32, tag="qp")
                nc.vector.tensor_scalar(qp, qs, 0.0, None, mybir.AluOpType.not_equal)
                kz = smpool.tile([P, NBLK], F32, tag="kz")
                nc.vector.tensor_scalar(kz, ks, 0.0, None, mybir.AluOpType.is_equal)

                # w_h[m] = msk0*(1-kz)  (masked, key not all-zero -> exp(0)=1)
                # hn_w[m] = -msk0*kz    (masked, key all-zero -> exp(NEG)=0; =h-msk0)
                m0kz = smpool.tile([P, NBLK], F32, tag="m0kz")
                nc.vector.tensor_mul(m0kz, msk0col, kz)
                w_h = smpool.tile([P, NBLK], F32, tag="w_h")
                nc.vector.tensor_sub(w_h, msk0col, m0kz)
                w_h_bf = smpool.tile([P, NBLK], BF16, tag="w_h_bf")
                nc.vector.tensor_copy(w_h_bf, w_h)
                hn_w = smpool.tile([P, NBLK], F32, tag="hn_w")
                nc.vector.tensor_scalar_mul(hn_w, m0kz, -1.0)

                # hn scalar = sum_m hn_w[m]: [128,16]x[128,1] -> [16,1] -> [1,1] -> [128,1]
                hn_ps16 = ps_sm.tile([NBLK, 1], F32, tag="sm")
                nc.tensor.matmul(hn_ps16, hn_w, ones_col, start=True, stop=True)
                hn_sb16 = smpool.tile([NBLK, 1], F32, tag="hn_sb16")
                nc.vector.tensor_copy(hn_sb16, hn_ps16)
                hn_ps1 = ps_sm.tile([1, 1], F32, tag="sm")
                nc.tensor.matmul(hn_ps1, hn_sb16, ones_col[:NBLK, :], start=True, stop=True)
                hn_tot = smpool.tile([1, 1], F32, tag="hn_tot")
                nc.vector.tensor_copy(hn_tot, hn_ps1)
                hn_ps128 = ps_sm.tile([P, 1], F32, tag="sm")
                nc.tensor.matmul(hn_ps128, ones_row, hn_tot, start=True, stop=True)
                hn128 = smpool.tile([P, 1], F32, tag="hn128")
                nc.vector.tensor_copy(hn128, hn_ps128)

                # ---------------- transposes QT/KT ----------------
                qT = qktpool.tile([P, NBLK, P], F32R, tag="qT")
                kT = qktpool.tile([P, NBLK, P], F32R, tag="kT")
                for a in range(NBLK):
                    t_ps = ps_sm.tile([P, P], F32, tag="sm")
                    nc.tensor.transpose(t_ps, k_sb[:, a, :], ident)
                    if a % 2 == 0:
                        nc.vector.tensor_copy(kT[:, a, :], t_ps)
                    else:
                        nc.scalar.copy(kT[:, a, :], t_ps)
                for a in range(NBLK):
                    t_ps = ps_sm.tile([P, P], F32, tag="sm")
                    nc.tensor.transpose(t_ps, q_sb[:, a, :], ident)
                    if a % 2 == 0:
                        nc.vector.tensor_copy(qT[:, a, :], t_ps)
                    else:
                        nc.scalar.copy(qT[:, a, :], t_ps)

                # ---------------- projections ----------------
                # eqT[d',n] scaled by 1/sqrt(D); ekT[d',m] masked by sel
                eqT = projpool.tile([P, N], F32R, tag="eqT")
                ekT = projpool.tile([P, N], F32R, tag="ekT")
                for c in range(4):
                    ps = ps_big.tile([P, 512], F32, tag="big")
                    nc.tensor.matmul(
                        ps,
                        wts["wk"],
                        kT[:, 4 * c : 4 * c + 4, :],
                        start=True,
                        stop=True,
                    )
                    nc.vector.tensor_scalar_add(
                        ekT[:, 512 * c : 512 * (c + 1)], ps, bk_c
                    )
                    # selb chunk via K=1 matmul -> ACT copy -> DVE multiply
                    selb_ps = ps_big.tile([P, 512], F32, tag="big")
                    nc.tensor.matmul(
                        selb_ps,
                        ones_row,
                        selrow[:, 512 * c : 512 * (c + 1)],
                        start=True,
                        stop=True,
                    )
                    nc.vector.tensor_mul(
                        ekT[:, 512 * c : 512 * (c + 1)],
                        ekT[:, 512 * c : 512 * (c + 1)],
                        selb_ps,
                    )

                for c in range(4):
                    ps = ps_big.tile([P, 512], F32, tag="big")
                    nc.tensor.matmul(
                        ps,
                        wts["wq"],
                        qT[:, 4 * c : 4 * c + 4, :],
                        start=True,
                        stop=True,
                    )
                    nc.vector.tensor_scalar(
                        eqT[:, 512 * c : 512 * (c + 1)],
                        ps,
                        bq_c,
                        SCALE,
                        mybir.AluOpType.add,
                        mybir.AluOpType.mult,
                    )
                # ev natural [m, d'], bias via K=1 rank-1; evq = ev*qp ; evz = evq*sel
                evq = evpool.tile([P, NBLK, P], BF16, tag="evq")
                evz = evpool.tile([P, NBLK, P], BF16, tag="evz")
                for a in range(NBLK):
                    ps = ps_sm.tile([P, P], F32, tag="sm")
                    nc.tensor.matmul(
                        ps, kT[:, a, :], wts["wv"], start=True, stop=False
                    )
                    nc.tensor.matmul(ps, ones_row, bv_r, start=False, stop=True)
                    nc.vector.tensor_scalar_mul(evq[:, a, :], ps, qp[:, a : a + 1])
                    nc.vector.tensor_scalar_mul(
                        evz[:, a, :], evq[:, a, :], selcol[:, a : a + 1]
                    )

                # hvec[d'] = sum_m w_h[m]*evq[m,d']
                hv_ps = ps_sm.tile([1, P], F32, tag="sm")
                for a in range(NBLK):
                    nc.tensor.matmul(
                        hv_ps,
                        w_h_bf[:, a : a + 1],
                        evq[:, a, :],
                        start=(a == 0),
                        stop=(a == NBLK - 1),
                    )
                hv_row = smpool.tile([1, P], F32, tag="hv_row")
                nc.vector.tensor_copy(hv_row, hv_ps)

                # ---------------- scores + exp + transpose ----------------
                recip = smpool.tile([P, NBLK], F32, tag="recip")
                pts = []
                for i in range(NBLK):
                    pb = ppool.tile([P, N], BF16, tag="pblk")
                    acc = smpool.tile([P, 2], F32, tag="acc")
                    for h in range(2):
                        ps = ps_big.tile([P, 1024], F32, tag="big")
                        for c in range(2):
                            nc.tensor.matmul(
                                ps[:, 512 * c : 512 * (c + 1)],
                                eqT[:, P * i : P * (i + 1)],
                                ekT[:, 1024 * h + 512 * c : 1024 * h + 512 * (c + 1)],
                                start=True,
                                stop=True,
                            )
                        nc.scalar.activation(
                            pb[:, 1024 * h : 1024 * (h + 1)],
                            ps,
                            mybir.ActivationFunctionType.Exp,
                            accum_out=acc[:, h : h + 1],
                        )
                    # rowsum = acc0+acc1+hn ; recip
                    rs = smpool.tile([P, 1], F32, tag="rs")
                    nc.vector.tensor_add(rs, acc[:, 0:1], acc[:, 1:2])
                    nc.vector.tensor_add(rs, rs, hn128)
                    nc.vector.reciprocal(recip[:, i : i + 1], rs)
                    # transpose probs row-block into pt_i[:, j, :]
                    pt_i = ptpool.tile([P, NBLK, P], BF16, tag="pt")
                    pts.append(pt_i)
                    for j in range(NBLK):
                        nc.sync.dma_start_transpose(
                            pt_i[:, j, :], pb[:, P * j : P * (j + 1)]
                        )

                # ---------------- AV + output ----------------
                for i in range(NBLK):
                    ps = ps_sm.tile([P, P], F32, tag="sm")
                    for j in range(NBLK):
                        nc.tensor.matmul(
                            ps,
                            pts[i][:, j, :],
                            evz[:, j, :],
                            start=(j == 0),
                            stop=False,
                        )
                    nc.tensor.matmul(ps, ones_row, hv_row, start=False, stop=True)
                    o_sb = opool.tile([P, P], F32, tag="o_sb")
                    nc.vector.tensor_scalar_mul(o_sb, ps, recip[:, i : i + 1])
                    nc.vector.tensor_add(o_sb, o_sb, q_sb[:, i, :])
                    nc.gpsimd.dma_start(
                        o_d[b, P * i : P * (i + 1), :], o_sb
                    )

    return nc


def kernel(queries, keys, padding_mask, Wq, bq, Wk, bk, Wv, bv):
    queries = np.ascontiguousarray(np.asarray(queries, dtype=np.float32))
    keys = np.ascontiguousarray(np.asarray(keys, dtype=np.float32))
    padding_mask = np.ascontiguousarray(np.asarray(padding_mask, dtype=np.int32))
    shared = {
        "Wq": np.ascontiguousarray(np.asarray(Wq, np.float32)),
        "Wk": np.ascontiguousarray(np.asarray(Wk, np.float32)),
        "Wv": np.ascontiguousarray(np.asarray(Wv, np.float32)),
        "bq": np.ascontiguousarray(np.asarray(bq, np.float32)),
        "bk": np.ascontiguousarray(np.asarray(bk, np.float32)),
        "bv": np.ascontiguousarray(np.asarray(bv, np.float32)),
    }
    if "nc" not in _NC_CACHE:
        nc0 = build_nc()
        if not nc0.is_finalized():
            nc0.finalize()
        _NC_CACHE["nc"] = nc0
    nc = _NC_CACHE["nc"]

    in_maps = []
    for c in range(NCORES):
        sl = slice(c * BPC, (c + 1) * BPC)
        in_maps.append(
            {
                "queries": queries[sl],
                "keys": keys[sl],
                "padding_mask": padding_mask[sl],
                **shared,
            }
        )
    res = bass_utils.run_bass_kernel_spmd(
        nc,
        in_maps,
        core_ids=list(range(NCORES)),
        trace=bool(int(os.environ.get("KERNEL_TRACE", "0"))),
    )
    out = np.concatenate([r["out"] for r in res.results], axis=0)
    _NC_CACHE["last_exec_time_ns"] = res.exec_time_ns
    _NC_CACHE["last_profile"] = res.profile_json
    return out
